# revision 13
# baseline (speedup 1.0000x reference)
"""Single-head attention (B=4, S=2048, D=1024) on 8 Trainium2 NeuronCores.

Sharding: batch x KEY-half. Core c handles batch b=c//2 and key rows
[1024*h : 1024*(h+1)] with h=c%2. Each core receives x[b] rolled so its own
key rows come first; it computes Q for ALL 2048 (rolled) queries, K/V for its
1024 keys, and outputs the UNNORMALIZED partial attention O~ = exp(S)V plus
partial row-sums. The host un-rolls the query order and combines the pair:
O = (O~_0 + O~_1) / (rs_0 + rs_1).  (No softmax max-subtraction is needed:
scaled scores are ~N(0,1), so exp never overflows, and partials add.)

v2 vs baseline:
  - All DRAM inputs declared float32r (same bits as f32) -> plain HWDGE
    DMAs everywhere, no SWDGE casting / DVE rounding copies.
  - Q^T stays fully resident in SBUF (8 MB) -- the 16 MB DRAM spill
    round-trip is gone, phase C has no input DMA dependencies.
  - x streams in 512-column chunks ordered so B1's first accumulation
    group unlocks after ~2.5 MB instead of the full 8 MB + weights.
  - Scores computed per 512-query tile (moving dim 512 everywhere).
  - Tile pools released phase-by-phase so peak SBUF stays under budget.

Per-core pipeline (activations kept [feature, token] transposed so the PE
contracts over partitions):
  B1: Q^T = Wq^T x^T + bq (all 2048 queries) -> resident [e, q]
  B2: K^T (own 1024 keys) -> resident [e, k]
  B3: V natural [k, e] (own keys) -> resident (bias via rank-1 ones x bv mm)
  C:  per 512-query tile: S^T[k, q] = K^T.T @ Q^T in transposed layout ->
      exp(scale*s) on ACT writes P^T straight to SBUF -> O~ = P^T.T @ V
      -> DMA out per 512-col chunk -> row-sums via ones-vector matmuls.
Matmuls run as float32r (1 cycle/row at N>=256 vs 4 for fp32).
"""

import sys
from contextlib import ExitStack

import numpy as np

if "/opt/trn_rl_repo" not in sys.path:
    sys.path.insert(0, "/opt/trn_rl_repo")

import concourse.bass as bass
import concourse.bacc as bacc
import concourse.tile as tile
from concourse import mybir
from concourse.bass_utils import run_bass_kernel_spmd

P = 128
S = 2048        # full sequence (queries per core)
SK = 1024       # keys per core (own half)
D = 1024        # model dim
F32 = mybir.dt.float32
F32R = mybir.dt.float32r

DC = D // P     # 8 d-chunks (contraction over model dim)
EC = D // P     # 8 e-chunks (output features)
KC = SK // P    # 8 key chunks (own half)
NT = 512        # moving-operand tile (one PSUM bank of fp32)

SCALE = 1.0 / float(np.sqrt(np.float32(D)))


def build_program() -> bass.Bass:
    nc = bacc.Bacc(
        "TRN2", target_bir_lowering=False, debug=False, num_devices=8)

    def _in(name, shape, dt=F32R):
        return nc.dram_tensor(name, shape, dt, kind="ExternalInput").ap()

    xT_d = _in("xT", [D, S])
    wq_d = _in("Wq", [D, D])
    bq_d = _in("bq", [D], F32)
    wk_d = _in("Wk", [D, D])
    bk_d = _in("bk", [D], F32)
    wv_d = _in("Wv", [D, D])
    bv_d = _in("bv", [D])
    o_d = nc.dram_tensor("o_raw", [S, D], F32, kind="ExternalOutput").ap()
    rs_d = nc.dram_tensor("rs_raw", [S], F32, kind="ExternalOutput").ap()

    with tile.TileContext(nc) as tc, ExitStack() as ctx:
        const_p = ctx.enter_context(tc.tile_pool(name="const", bufs=1))
        qsb_p = ctx.enter_context(tc.tile_pool(name="qsb", bufs=EC))
        psB = ctx.enter_context(tc.tile_pool(name="psB", bufs=3, space="PSUM"))
        psO = ctx.enter_context(tc.tile_pool(name="psO", bufs=3, space="PSUM"))
        psA = ctx.enter_context(tc.tile_pool(name="psA", bufs=2, space="PSUM"))

        # ---- constants -------------------------------------------------
        bqt = const_p.tile([P, EC], F32)  # bq chunked [p, ec]
        nc.sync.dma_start(bqt[:], bq_d[:].rearrange("(c p) -> p c", p=P))
        bkt = const_p.tile([P, EC], F32)
        nc.sync.dma_start(bkt[:], bk_d[:].rearrange("(c p) -> p c", p=P))
        bvr = const_p.tile([1, D], F32R)  # bv as a single row
        nc.sync.dma_start(bvr[:], bv_d[:].rearrange("(o d) -> o d", o=1))
        ones_raw = const_p.tile([P, 1], F32)
        nc.vector.memset(ones_raw[:], 1.0)
        ones = const_p.tile([P, 1], F32R)  # column of ones: lhsT for row-sums
        nc.vector.tensor_copy(ones[:], ones_raw[:])
        onesr_raw = const_p.tile([1, P], F32)
        nc.vector.memset(onesr_raw[:], 1.0)
        ones_row = const_p.tile([1, P], F32R)  # row of ones: V-bias rank-1 mm
        nc.vector.tensor_copy(ones_row[:], onesr_raw[:])

        # Q^T resident for all 2048 queries
        qsb = [qsb_p.tile([P, S], F32R, name=f"q{ec}", tag="qsb")
               for ec in range(EC)]

        with tc.tile_pool(name="xtA", bufs=DC) as xtA_p:
            xtA = [xtA_p.tile([P, SK], F32R, name=f"xtA{dc}", tag="xt")
                   for dc in range(DC)]

            with tc.tile_pool(name="wqk", bufs=16) as wqk_p:
                with tc.tile_pool(name="xtB", bufs=DC) as xtB_p:
                    xtB = [xtB_p.tile([P, SK], F32R, name=f"xtB{dc}",
                                      tag="xt")
                           for dc in range(DC)]
                    xt = [xtA, xtB]
                    # x^T streamed in 512-col chunks, column-major so B1's
                    # first accumulation group unlocks after ~2.5 MB
                    for qt_i in range(S // NT):
                        h, sh = divmod(qt_i, 2)
                        for dc in range(DC):
                            nc.sync.dma_start(
                                xt[h][dc][:, sh * NT:(sh + 1) * NT],
                                xT_d[dc * P:(dc + 1) * P,
                                     qt_i * NT:(qt_i + 1) * NT])

                    # weights as natural [128, 1024] row tiles (4 KB
                    # contiguous DMA lines); stationary = w[dc][:, ec*128:]
                    # loaded in 512-col halves so the first B1 chain
                    # unlocks after 2 MB
                    wq = [wqk_p.tile([P, D], F32R, name=f"wq{dc}",
                                     tag="wqk") for dc in range(DC)]
                    wk = [wqk_p.tile([P, D], F32R, name=f"wk{dc}",
                                     tag="wqk") for dc in range(DC)]
                    for half in range(2):
                        for dc in range(DC):
                            nc.gpsimd.dma_start(
                                wq[dc][:, half * NT:(half + 1) * NT],
                                wq_d[dc * P:(dc + 1) * P,
                                     half * NT:(half + 1) * NT])
                    for half in range(2):
                        for dc in range(DC):
                            nc.gpsimd.dma_start(
                                wk[dc][:, half * NT:(half + 1) * NT],
                                wk_d[dc * P:(dc + 1) * P,
                                     half * NT:(half + 1) * NT])

                    # ---- Phase B1: Q^T (all queries) resident ----------
                    for qt_i in range(S // NT):
                        h, sh = divmod(qt_i, 2)
                        for ec in range(EC):
                            ps = psB.tile([P, NT], F32)
                            for dc in range(DC):
                                nc.tensor.matmul(
                                    ps[:],
                                    (wq[dc][:, ec * P:(ec + 1) * P]),
                                    (xt[h][dc][:, sh * NT:(sh + 1) * NT]),
                                    start=(dc == 0), stop=(dc == DC - 1),
                                )
                            nc.scalar.activation(
                                qsb[ec][:, qt_i * NT:(qt_i + 1) * NT],
                                ps[:],
                                mybir.ActivationFunctionType.Identity,
                                bias=bqt[:, ec:ec + 1],
                            )

                # ---- Phase B2: K^T (own keys) resident -----------------
                kt_p = ctx.enter_context(tc.tile_pool(name="kt", bufs=EC, side="right"))
                kt = [kt_p.tile([P, SK], F32R, name=f"kt{ec}", tag="kt")
                      for ec in range(EC)]
                for kt_i in range(SK // NT):
                    for ec in range(EC):
                        ps = psB.tile([P, NT], F32)
                        for dc in range(DC):
                            nc.tensor.matmul(
                                ps[:],
                                (wk[dc][:, ec * P:(ec + 1) * P]),
                                (xtA[dc][:, kt_i * NT:(kt_i + 1) * NT]),
                                start=(dc == 0), stop=(dc == DC - 1),
                            )
                        nc.scalar.activation(
                            kt[ec][:, kt_i * NT:(kt_i + 1) * NT], ps[:],
                            mybir.ActivationFunctionType.Identity,
                            bias=bkt[:, ec:ec + 1],
                        )

            # ---- Phase B3: V natural [k, e] (own keys) resident --------
            v_p = ctx.enter_context(tc.tile_pool(name="v", bufs=KC, side="right"))
            v = [v_p.tile([P, D], F32R, name=f"v{kc}", tag="v")
                 for kc in range(KC)]
            with tc.tile_pool(name="wv", bufs=16) as wv_p:
                wvt = [[wv_p.tile([P, NT], F32R, name=f"wvh{et}_{dc}",
                                  tag="wv")
                        for dc in range(DC)] for et in range(D // NT)]
                for et in range(D // NT):
                    for dc in range(DC):
                        nc.sync.dma_start(
                            wvt[et][dc][:],
                            wv_d[dc * P:(dc + 1) * P,
                                 et * NT:(et + 1) * NT])
                for et in range(D // NT):
                    wvh = wvt[et]
                    for kc in range(KC):
                        ps = psB.tile([P, NT], F32)
                        for dc in range(DC):
                            nc.tensor.matmul(
                                ps[:],
                                (xtA[dc][:, kc * P:(kc + 1) * P]),
                                (wvh[dc][:]),
                                start=(dc == 0), stop=False,
                            )
                        # rank-1 bias add: ones_row^T @ bv_row
                        nc.tensor.matmul(
                            ps[:],
                            (ones_row[0:1, :]),
                            (bvr[0:1, et * NT:(et + 1) * NT]),
                            start=False, stop=True,
                        )
                        nc.vector.tensor_copy(
                            v[kc][:, et * NT:(et + 1) * NT], ps[:])

        # ---- Phase C: attention, transposed scores, 512-query tiles ----
        io_p = ctx.enter_context(tc.tile_pool(name="io", bufs=4, side="right"))
        st_p = ctx.enter_context(tc.tile_pool(name="stat", bufs=2, side="right"))
        with tc.tile_pool(name="ptp", bufs=2 * KC) as pt_p:
            for qh in range(S // NT):
                # S^T[k, q] per key chunk; exp writes P^T straight to SBUF
                ptt = [pt_p.tile([P, NT], F32R, tag="ptp", name=f"pt{kc}")
                       for kc in range(KC)]
                for kc in range(KC):
                    ps = psB.tile([P, NT], F32)
                    for ec in range(EC):
                        nc.tensor.matmul(
                            ps[:],
                            (kt[ec][:, kc * P:(kc + 1) * P]),
                            (qsb[ec][:, qh * NT:(qh + 1) * NT]),
                            start=(ec == 0), stop=(ec == EC - 1),
                        )
                    nc.scalar.activation(
                        ptt[kc][:], ps[:],
                        mybir.ActivationFunctionType.Exp,
                        scale=SCALE,
                    )

                # O~ = P^T.T @ V per 128-query chunk; stationary (ptt slice)
                # reused across the two 512-col output chunks
                for qc in range(NT // P):
                    pso = [psO.tile([P, NT], F32, name="pso")
                           for _ in range(D // NT)]
                    for kc in range(KC):
                        for et in range(D // NT):
                            nc.tensor.matmul(
                                pso[et][:],
                                (ptt[kc][:, qc * P:(qc + 1) * P]),
                                (v[kc][:, et * NT:(et + 1) * NT]),
                                start=(kc == 0), stop=(kc == KC - 1),
                            )
                    row0 = qh * NT + qc * P
                    for et in range(D // NT):
                        o_sb = io_p.tile([P, NT], F32, name="osb", tag="io")
                        if et == 0:
                            nc.vector.tensor_copy(o_sb[:], pso[et][:])
                        else:
                            nc.scalar.activation(
                                o_sb[:], pso[et][:],
                                mybir.ActivationFunctionType.Identity)
                        nc.sync.dma_start(
                            o_d[row0:row0 + P, et * NT:(et + 1) * NT],
                            o_sb[:])

                # partial row-sums: ones^T @ P^T, accumulated over key chunks
                ps_rs = psA.tile([1, NT], F32, name="ps_rs")
                for kc in range(KC):
                    nc.tensor.matmul(
                        ps_rs[:],
                        (ones[:, 0:1]),
                        (ptt[kc][:]),
                        start=(kc == 0), stop=(kc == KC - 1),
                    )
                rs_sb = st_p.tile([1, NT], F32, name="rs_sb", tag="rs")
                nc.vector.tensor_copy(rs_sb[:], ps_rs[:])
                nc.sync.dma_start(
                    rs_d[qh * NT:(qh + 1) * NT].rearrange(
                        "(o q) -> o q", o=1),
                    rs_sb[:])

    nc.compile()
    return nc


_CACHE: dict = {}


def _get_program() -> bass.Bass:
    if "nc" not in _CACHE:
        _CACHE["nc"] = build_program()
    return _CACHE["nc"]


def kernel(x, Wq, bq, Wk, bk, Wv, bv, _trace=False, _trace_kwargs=None):
    nc = _get_program()
    x = np.asarray(x, dtype=np.float32)
    shared = {
        "Wq": np.ascontiguousarray(np.asarray(Wq, np.float32)),
        "bq": np.ascontiguousarray(np.asarray(bq, np.float32)),
        "Wk": np.ascontiguousarray(np.asarray(Wk, np.float32)),
        "bk": np.ascontiguousarray(np.asarray(bk, np.float32)),
        "Wv": np.ascontiguousarray(np.asarray(Wv, np.float32)),
        "bv": np.ascontiguousarray(np.asarray(bv, np.float32)),
    }
    in_maps = []
    for c in range(8):
        b, h = divmod(c, 2)
        xb = x[b]
        if h:
            xb = np.roll(xb, -SK, axis=0)  # own key half first
        in_maps.append(
            {"xT": np.ascontiguousarray(xb.T), **shared})

    res = run_bass_kernel_spmd(
        nc, in_maps, list(range(8)),
        trace=_trace, **(_trace_kwargs or {}),
    )
    out = np.empty((4, S, D), dtype=np.float32)
    for b in range(4):
        o0 = res.results[2 * b]["o_raw"].astype(np.float64)
        r0 = res.results[2 * b]["rs_raw"].astype(np.float64)
        o1 = res.results[2 * b + 1]["o_raw"].astype(np.float64)
        r1 = res.results[2 * b + 1]["rs_raw"].astype(np.float64)
        # core h=1 computed queries in rolled order; un-roll before combining
        o1 = np.roll(o1, SK, axis=0)
        r1 = np.roll(r1, SK)
        out[b] = ((o0 + o1) / (r0 + r1)[:, None]).astype(np.float32)
    if _trace:
        return out, res
    return out


# revision 15
# speedup vs baseline: 1.1846x; 1.1846x over previous
"""Single-head attention (B=4, S=2048, D=1024) on 8 Trainium2 NeuronCores.

Sharding: batch x KEY-half. Core c handles batch b=c//2 and key rows
[1024*h : 1024*(h+1)] with h=c%2. Each core receives x[b] rolled so its own
key rows come first; it computes Q for ALL 2048 (rolled) queries, K/V for its
1024 keys, and outputs the UNNORMALIZED partial attention O~ = exp(S)V plus
partial row-sums. The host un-rolls the query order and combines the pair:
O = (O~_0 + O~_1) / (rs_0 + rs_1).  (No softmax max-subtraction is needed:
scaled scores are ~N(0,1), so exp never overflows, and partials add.)

v2 vs baseline:
  - All DRAM inputs declared float32r (same bits as f32) -> plain HWDGE
    DMAs everywhere, no SWDGE casting / DVE rounding copies.
  - Q^T stays fully resident in SBUF (8 MB) -- the 16 MB DRAM spill
    round-trip is gone, phase C has no input DMA dependencies.
  - x streams in 512-column chunks ordered so B1's first accumulation
    group unlocks after ~2.5 MB instead of the full 8 MB + weights.
  - Scores computed per 512-query tile (moving dim 512 everywhere).
  - Tile pools released phase-by-phase so peak SBUF stays under budget.

Per-core pipeline (activations kept [feature, token] transposed so the PE
contracts over partitions):
  B1: Q^T = Wq^T x^T + bq (all 2048 queries) -> resident [e, q]
  B2: K^T (own 1024 keys) -> resident [e, k]
  B3: V natural [k, e] (own keys) -> resident (bias via rank-1 ones x bv mm)
  C:  per 512-query tile: S^T[k, q] = K^T.T @ Q^T in transposed layout ->
      exp(scale*s) on ACT writes P^T straight to SBUF -> O~ = P^T.T @ V
      -> DMA out per 512-col chunk -> row-sums via ones-vector matmuls.
Matmuls run as float32r (1 cycle/row at N>=256 vs 4 for fp32).
"""

import sys
from contextlib import ExitStack

import numpy as np

if "/opt/trn_rl_repo" not in sys.path:
    sys.path.insert(0, "/opt/trn_rl_repo")

import concourse.bass as bass
import concourse.bacc as bacc
import concourse.tile as tile
from concourse import mybir
from concourse.bass_utils import run_bass_kernel_spmd

P = 128
S = 2048        # full sequence (queries per core)
SK = 1024       # keys per core (own half)
D = 1024        # model dim
F32 = mybir.dt.float32
F32R = mybir.dt.float32r

DC = D // P     # 8 d-chunks (contraction over model dim)
EC = D // P     # 8 e-chunks (output features)
KC = SK // P    # 8 key chunks (own half)
NT = 512        # moving-operand tile (one PSUM bank of fp32)

SCALE = 1.0 / float(np.sqrt(np.float32(D)))


def build_program() -> bass.Bass:
    nc = bacc.Bacc(
        "TRN2", target_bir_lowering=False, debug=False, num_devices=8)

    def _in(name, shape, dt=F32R):
        return nc.dram_tensor(name, shape, dt, kind="ExternalInput").ap()

    xT_d = _in("xT", [D, S])
    wq_d = _in("Wq", [D, D])
    bq_d = _in("bq", [D], F32)
    wk_d = _in("Wk", [D, D])
    bk_d = _in("bk", [D], F32)
    wv_d = _in("Wv", [D, D])
    bv_d = _in("bv", [D])
    o_d = nc.dram_tensor("o_raw", [S, D], F32, kind="ExternalOutput").ap()
    rs_d = nc.dram_tensor("rs_raw", [S], F32, kind="ExternalOutput").ap()

    with tile.TileContext(nc) as tc, ExitStack() as ctx:
        const_p = ctx.enter_context(tc.tile_pool(name="const", bufs=1))
        qsb_p = ctx.enter_context(tc.tile_pool(name="qsb", bufs=EC))
        psB = ctx.enter_context(tc.tile_pool(name="psB", bufs=3, space="PSUM"))
        psO = ctx.enter_context(tc.tile_pool(name="psO", bufs=3, space="PSUM"))
        psA = ctx.enter_context(tc.tile_pool(name="psA", bufs=2, space="PSUM"))

        # ---- constants -------------------------------------------------
        bqt = const_p.tile([P, EC], F32)  # bq chunked [p, ec]
        nc.sync.dma_start(bqt[:], bq_d[:].rearrange("(c p) -> p c", p=P))
        bkt = const_p.tile([P, EC], F32)
        nc.sync.dma_start(bkt[:], bk_d[:].rearrange("(c p) -> p c", p=P))
        bvr = const_p.tile([1, D], F32R)  # bv as a single row
        nc.sync.dma_start(bvr[:], bv_d[:].rearrange("(o d) -> o d", o=1))
        ones_raw = const_p.tile([P, 1], F32)
        nc.vector.memset(ones_raw[:], 1.0)
        ones = const_p.tile([P, 1], F32R)  # column of ones: lhsT for row-sums
        nc.vector.tensor_copy(ones[:], ones_raw[:])
        onesr_raw = const_p.tile([1, P], F32)
        nc.vector.memset(onesr_raw[:], 1.0)
        ones_row = const_p.tile([1, P], F32R)  # row of ones: V-bias rank-1 mm
        nc.vector.tensor_copy(ones_row[:], onesr_raw[:])

        # Q^T resident for all 2048 queries
        qsb = [qsb_p.tile([P, S], F32R, name=f"q{ec}", tag="qsb")
               for ec in range(EC)]

        with tc.tile_pool(name="xtA", bufs=DC) as xtA_p:
            xtA = [xtA_p.tile([P, SK], F32R, name=f"xtA{dc}", tag="xt")
                   for dc in range(DC)]

            with tc.tile_pool(name="wqk", bufs=16) as wqk_p:
                with tc.tile_pool(name="xtB", bufs=DC) as xtB_p:
                    xtB = [xtB_p.tile([P, SK], F32R, name=f"xtB{dc}",
                                      tag="xt")
                           for dc in range(DC)]
                    xt = [xtA, xtB]
                    # x^T streamed in 512-col chunks, column-major so B1's
                    # first accumulation group unlocks after ~2.5 MB
                    for qt_i in range(S // NT):
                        h, sh = divmod(qt_i, 2)
                        for dc in range(DC):
                            nc.sync.dma_start(
                                xt[h][dc][:, sh * NT:(sh + 1) * NT],
                                xT_d[dc * P:(dc + 1) * P,
                                     qt_i * NT:(qt_i + 1) * NT])

                    # weights as natural [128, 1024] row tiles (4 KB
                    # contiguous DMA lines); stationary = w[dc][:, ec*128:]
                    # loaded in 512-col halves so the first B1 chain
                    # unlocks after 2 MB
                    wq = [wqk_p.tile([P, D], F32R, name=f"wq{dc}",
                                     tag="wqk") for dc in range(DC)]
                    wk = [wqk_p.tile([P, D], F32R, name=f"wk{dc}",
                                     tag="wqk") for dc in range(DC)]
                    # halves split across the two spare trigger queues
                    # (scalar HWDGE + gpsimd SWDGE) so full Wq lands ~2x
                    # faster; all dst tiles are fresh slots, so the scalar
                    # triggers never wait and can't block B1's ACT stream
                    for w, w_d in ((wq, wq_d), (wk, wk_d)):
                        for dc in range(DC):
                            nc.scalar.dma_start(
                                w[dc][:, 0:NT],
                                w_d[dc * P:(dc + 1) * P, 0:NT])
                        for dc in range(DC):
                            nc.gpsimd.dma_start(
                                w[dc][:, NT:D],
                                w_d[dc * P:(dc + 1) * P, NT:D])

                    # ---- Phase B1: Q^T (all queries) resident ----------
                    for qt_i in range(S // NT):
                        h, sh = divmod(qt_i, 2)
                        for ec in range(EC):
                            ps = psB.tile([P, NT], F32)
                            for dc in range(DC):
                                nc.tensor.matmul(
                                    ps[:],
                                    (wq[dc][:, ec * P:(ec + 1) * P]),
                                    (xt[h][dc][:, sh * NT:(sh + 1) * NT]),
                                    start=(dc == 0), stop=(dc == DC - 1),
                                )
                            nc.scalar.activation(
                                qsb[ec][:, qt_i * NT:(qt_i + 1) * NT],
                                ps[:],
                                mybir.ActivationFunctionType.Identity,
                                bias=bqt[:, ec:ec + 1],
                            )

                # ---- Phase B2: K^T (own keys) resident -----------------
                kt_p = ctx.enter_context(tc.tile_pool(name="kt", bufs=EC, side="right"))
                kt = [kt_p.tile([P, SK], F32R, name=f"kt{ec}", tag="kt")
                      for ec in range(EC)]
                for kt_i in range(SK // NT):
                    for ec in range(EC):
                        ps = psB.tile([P, NT], F32)
                        for dc in range(DC):
                            nc.tensor.matmul(
                                ps[:],
                                (wk[dc][:, ec * P:(ec + 1) * P]),
                                (xtA[dc][:, kt_i * NT:(kt_i + 1) * NT]),
                                start=(dc == 0), stop=(dc == DC - 1),
                            )
                        nc.scalar.activation(
                            kt[ec][:, kt_i * NT:(kt_i + 1) * NT], ps[:],
                            mybir.ActivationFunctionType.Identity,
                            bias=bkt[:, ec:ec + 1],
                        )

            # ---- Phase B3: V natural [k, e] (own keys) resident --------
            v_p = ctx.enter_context(tc.tile_pool(name="v", bufs=KC, side="right"))
            v = [v_p.tile([P, D], F32R, name=f"v{kc}", tag="v")
                 for kc in range(KC)]
            with tc.tile_pool(name="wv", bufs=16) as wv_p:
                wvt = [[wv_p.tile([P, NT], F32R, name=f"wvh{et}_{dc}",
                                  tag="wv")
                        for dc in range(DC)] for et in range(D // NT)]
                for et in range(D // NT):
                    for dc in range(DC):
                        nc.sync.dma_start(
                            wvt[et][dc][:],
                            wv_d[dc * P:(dc + 1) * P,
                                 et * NT:(et + 1) * NT])
                for et in range(D // NT):
                    wvh = wvt[et]
                    for kc in range(KC):
                        ps = psB.tile([P, NT], F32)
                        for dc in range(DC):
                            nc.tensor.matmul(
                                ps[:],
                                (xtA[dc][:, kc * P:(kc + 1) * P]),
                                (wvh[dc][:]),
                                start=(dc == 0), stop=False,
                            )
                        # rank-1 bias add: ones_row^T @ bv_row
                        nc.tensor.matmul(
                            ps[:],
                            (ones_row[0:1, :]),
                            (bvr[0:1, et * NT:(et + 1) * NT]),
                            start=False, stop=True,
                        )
                        nc.vector.tensor_copy(
                            v[kc][:, et * NT:(et + 1) * NT], ps[:])

        # ---- Phase C: attention, transposed scores, 512-query tiles ----
        io_p = ctx.enter_context(tc.tile_pool(name="io", bufs=4, side="right"))
        st_p = ctx.enter_context(tc.tile_pool(name="stat", bufs=2, side="right"))
        with tc.tile_pool(name="ptp", bufs=2 * KC) as pt_p:
            for qh in range(S // NT):
                # S^T[k, q] per key chunk; exp writes P^T straight to SBUF
                ptt = [pt_p.tile([P, NT], F32R, tag="ptp", name=f"pt{kc}")
                       for kc in range(KC)]
                for kc in range(KC):
                    ps = psB.tile([P, NT], F32)
                    for ec in range(EC):
                        nc.tensor.matmul(
                            ps[:],
                            (kt[ec][:, kc * P:(kc + 1) * P]),
                            (qsb[ec][:, qh * NT:(qh + 1) * NT]),
                            start=(ec == 0), stop=(ec == EC - 1),
                        )
                    nc.scalar.activation(
                        ptt[kc][:], ps[:],
                        mybir.ActivationFunctionType.Exp,
                        scale=SCALE,
                    )

                # O~ = P^T.T @ V per 128-query chunk; stationary (ptt slice)
                # reused across the two 512-col output chunks
                for qc in range(NT // P):
                    pso = [psO.tile([P, NT], F32, name="pso")
                           for _ in range(D // NT)]
                    for kc in range(KC):
                        for et in range(D // NT):
                            nc.tensor.matmul(
                                pso[et][:],
                                (ptt[kc][:, qc * P:(qc + 1) * P]),
                                (v[kc][:, et * NT:(et + 1) * NT]),
                                start=(kc == 0), stop=(kc == KC - 1),
                            )
                    row0 = qh * NT + qc * P
                    for et in range(D // NT):
                        o_sb = io_p.tile([P, NT], F32, name="osb", tag="io")
                        nc.vector.tensor_copy(o_sb[:], pso[et][:])
                        nc.sync.dma_start(
                            o_d[row0:row0 + P, et * NT:(et + 1) * NT],
                            o_sb[:])

                # partial row-sums: ones^T @ P^T, accumulated over key chunks
                ps_rs = psA.tile([1, NT], F32, name="ps_rs")
                for kc in range(KC):
                    nc.tensor.matmul(
                        ps_rs[:],
                        (ones[:, 0:1]),
                        (ptt[kc][:]),
                        start=(kc == 0), stop=(kc == KC - 1),
                    )
                rs_sb = st_p.tile([1, NT], F32, name="rs_sb", tag="rs")
                nc.vector.tensor_copy(rs_sb[:], ps_rs[:])
                nc.sync.dma_start(
                    rs_d[qh * NT:(qh + 1) * NT].rearrange(
                        "(o q) -> o q", o=1),
                    rs_sb[:])

    nc.compile()
    return nc


_CACHE: dict = {}


def _get_program() -> bass.Bass:
    if "nc" not in _CACHE:
        _CACHE["nc"] = build_program()
    return _CACHE["nc"]


def kernel(x, Wq, bq, Wk, bk, Wv, bv, _trace=False, _trace_kwargs=None):
    nc = _get_program()
    x = np.asarray(x, dtype=np.float32)
    shared = {
        "Wq": np.ascontiguousarray(np.asarray(Wq, np.float32)),
        "bq": np.ascontiguousarray(np.asarray(bq, np.float32)),
        "Wk": np.ascontiguousarray(np.asarray(Wk, np.float32)),
        "bk": np.ascontiguousarray(np.asarray(bk, np.float32)),
        "Wv": np.ascontiguousarray(np.asarray(Wv, np.float32)),
        "bv": np.ascontiguousarray(np.asarray(bv, np.float32)),
    }
    in_maps = []
    for c in range(8):
        b, h = divmod(c, 2)
        xb = x[b]
        if h:
            xb = np.roll(xb, -SK, axis=0)  # own key half first
        in_maps.append(
            {"xT": np.ascontiguousarray(xb.T), **shared})

    res = run_bass_kernel_spmd(
        nc, in_maps, list(range(8)),
        trace=_trace, **(_trace_kwargs or {}),
    )
    out = np.empty((4, S, D), dtype=np.float32)
    for b in range(4):
        o0 = res.results[2 * b]["o_raw"].astype(np.float64)
        r0 = res.results[2 * b]["rs_raw"].astype(np.float64)
        o1 = res.results[2 * b + 1]["o_raw"].astype(np.float64)
        r1 = res.results[2 * b + 1]["rs_raw"].astype(np.float64)
        # core h=1 computed queries in rolled order; un-roll before combining
        o1 = np.roll(o1, SK, axis=0)
        r1 = np.roll(r1, SK)
        out[b] = ((o0 + o1) / (r0 + r1)[:, None]).astype(np.float32)
    if _trace:
        return out, res
    return out
